# revision 25
# baseline (speedup 1.0000x reference)
"""Causal multi-head attention with RoPE on 8 Trainium2 NeuronCores.

Problem: B=2, S=2048, D=1024, H=16 heads, head_dim=64, fp32 in/out.

Sharding (hardcoded): 8 cores = 2 (batch) x 4 (head groups of 4 heads).
Core c handles batch b = c // 4 and heads [hg*4, hg*4+4), hg = c % 4.
Each core computes its 4 heads' attention plus the partial output
projection o_part = attn_part @ wo[:, cols].T; the host sums the 4
partials per batch (the row-parallel reduction) to form the output.

Device dataflow per core (all matmuls in bf16, fp32 accumulation):
  qT/kT projections in transposed layout (channels on partitions),
  RoPE applied in that layout: channels of wq/wk are pre-permuted on
  host so each head's dims are [evens, odds]; the pair-rotation then
  needs the half-swapped vector, obtained with a 128x128 permutation
  matmul, and two elementwise multiplies against cos/sin tables.
  Scores are computed transposed, sT = k_rot @ q_rot.T (Sk on
  partitions), exp applied on ScalarE (scale=1/8 folded in), causal
  masking via memset + one triangular-mask multiply on the diagonal
  128-block. A@V uses exp(sT) blocks as the moving operand with
  stationary [v_h | ones] (M=65), so partition 64 of the accumulator
  carries the softmax denominators. Normalization broadcasts 1/r
  across partitions with an accumulating ones-matmul (hi+lo bf16
  split, so the broadcast is fp32-accurate), then the wo projection
  contracts the 256 channels and streams fp32 results to DRAM.

Scheduling: xT is DMAed in 32 [128,512] chunk tiles ordered so the
first seq chunk of every contraction tile lands first; weights are
DMAed as separate tiles in need order (wv first, wo last). v/qk
projection emission is interleaved chunk-by-chunk so the PE starts
~3us into the transfer instead of waiting for the full 4MB. RoPE
elementwise work runs all-bf16 on DVE (2x/4x perf modes); causal
tri-mask multiplies, PSUM->SBUF copies and output staging copies run
on the otherwise-idle Pool (gpsimd) engine so DVE/ScalarE keep up
with the PE during the attention phase.
"""

import numpy as np
import ml_dtypes

import concourse.bass as bass
import concourse.mybir as mybir
import concourse.tile as tile_mod
from concourse.bass_utils import run_bass_kernel_spmd

BF16 = ml_dtypes.bfloat16
dt = mybir.dt

B = 2
S = 2048
D = 1024
H = 16
HD = 64          # head dim
HPC = 4          # heads per core
NCH = HPC * HD   # 256 channels per core
KT = D // 128    # 8 contraction tiles over D
NM = S // 128    # 16 seq tiles of 128
NJ = S // 512    # 4 seq chunks of 512
THETA = 10000.0

_CACHE = {}

# Bumped on every kernel change: the Neuron compile cache hashes the HLO
# module WITHOUT the embedded BIR payload, so two different kernels with
# identical I/O signatures collide. A version-sized dummy input forces a
# distinct hash per kernel revision.
KVER = 17


def _split_multi_waits(nc):
    # This container's walrus build rejects >1 sync wait per instruction.
    # Hoist extra waits onto InstEventSemaphore carriers placed before the
    # instruction in the same engine's stream.
    for bb in nc.main_func.blocks:
        new_list = []
        for ins in bb.instructions:
            si = getattr(ins, "sync_info", None)
            if si is not None and si.on_wait and len(si.on_wait) > 1:
                waits = list(si.on_wait)
                si.on_wait = [waits[-1]]
                for w in waits[:-1]:
                    ev = mybir.InstEventSemaphore(
                        name=nc.get_next_instruction_name(),
                        engine=ins.engine,
                        ins=[],
                        outs=[],
                        sync_info=mybir.SyncInfo(on_wait=[w], on_update=[]),
                    )
                    nc.register_instruction(ev, overwrite=True)
                    new_list.append(ev)
            new_list.append(ins)
        bb.instructions[:] = new_list


def _build_nc():
    nc = bass.Bass("TRN2", target_bir_lowering=False)

    # Inputs are shipped in SBUF layout (128 partitions first).
    xT = nc.dram_tensor("xT", [KT * 128, S], dt.bfloat16, kind="ExternalInput")
    # packed bf16: wv|wq|wk (3*2048) | perm (128) | tri (128) | wo (2048)
    wpack = nc.dram_tensor("wpack", [128, 8448], dt.bfloat16, kind="ExternalInput")
    # packed bf16: cos (2048) | sin (2048) | ver pad (KVER)
    fpack = nc.dram_tensor("fpack", [128, 4096 + KVER], dt.bfloat16, kind="ExternalInput")
    out = nc.dram_tensor("o", [S, D], dt.float32, kind="ExternalOutput")

    EXP = mybir.ActivationFunctionType.Exp

    with tile_mod.TileContext(nc) as tc:
        with (
            tc.tile_pool(name="io", bufs=1) as io,
            tc.tile_pool(name="wk1", bufs=6) as wkp,
            tc.tile_pool(name="ep", bufs=8) as ep,
            tc.tile_pool(name="sm", bufs=6) as sm,
            tc.tile_pool(name="ob", bufs=4) as ob,
            tc.tile_pool(name="ps", bufs=2, space="PSUM") as ps,
            tc.tile_pool(name="pscr", bufs=2, space="PSUM") as scr_p,
            tc.tile_pool(name="po", bufs=2, space="PSUM") as po_p,
        ):
            # xT as 8x4 separate [128, 512] chunk tiles so dependencies are
            # exact per seq chunk; DMA issue order below puts chunk 0 of
            # every contraction tile first so compute starts early.
            xTc = [
                [io.tile([128, 512], dt.bfloat16, tag=f"xT{k}_{j}", name=f"xT{k}_{j}")
                 for j in range(NJ)] for k in range(KT)
            ]
            wv_sb = io.tile([128, 2048], dt.bfloat16, tag="wv")
            wq_sb = io.tile([128, 2048], dt.bfloat16, tag="wq")
            wk_sb = io.tile([128, 2048], dt.bfloat16, tag="wk")
            pt_sb = io.tile([128, 256], dt.bfloat16, tag="pt")
            wo_sb = io.tile([128, 2048], dt.bfloat16, tag="wo")
            fp_sb = io.tile([128, 4096 + KVER], dt.bfloat16, tag="fp")

            def dma_xt_chunk(j):
                for k in range(KT):
                    nc.sync.dma_start(
                        xTc[k][j][:], xT[k * 128 : (k + 1) * 128, j * 512 : (j + 1) * 512]
                    )

            # need-ordered input DMA: wv + chunk0 feed the first v/qk
            # matmuls ~3us in; wo arrives last (first needed ~60% through).
            nc.sync.dma_start(wv_sb[:], wpack[:, 0:2048])
            dma_xt_chunk(0)
            nc.sync.dma_start(wq_sb[:], wpack[:, 2048:4096])
            nc.sync.dma_start(wk_sb[:], wpack[:, 4096:6144])
            nc.sync.dma_start(pt_sb[:], wpack[:, 6144:6400])
            nc.sync.dma_start(fp_sb[:], fpack[:])
            dma_xt_chunk(1)
            dma_xt_chunk(2)
            dma_xt_chunk(3)
            nc.sync.dma_start(wo_sb[:], wpack[:, 6400:8448])

            perm_sb = pt_sb[:, 0:128]
            tri_sb = pt_sb[:, 128:256]
            cos_sb = fp_sb[:, 0:2048]
            sin_sb = fp_sb[:, 2048:4096]
            ones_sb = io.tile([1, 64], dt.bfloat16, tag="ones")
            nc.vector.memset(ones_sb[:], 1.0)

            # fine-grained persistent tiles: precise cross-phase dependencies
            q_t = [
                [io.tile([128, 512], dt.bfloat16, tag=f"q{g}{j}", name=f"q{g}{j}")
                 for j in range(NJ)] for g in range(2)
            ]
            k_t = [
                [io.tile([128, 512], dt.bfloat16, tag=f"k{g}{j}", name=f"k{g}{j}")
                 for j in range(NJ)] for g in range(2)
            ]
            v_t = [
                io.tile([128, HPC * 65], dt.bfloat16, tag=f"v{m}", name=f"v{m}")
                for m in range(NM)
            ]
            attn_t = [
                [io.tile([128, 512], dt.bfloat16, tag=f"at{g}{j}", name=f"at{g}{j}")
                 for j in range(NJ)] for g in range(2)
            ]

            def v_proj(m):
                pv = ps.tile([128, NCH], dt.float32, tag="ps", name="pv")
                mj, mo = divmod(m, 4)
                for k in range(KT):
                    nc.tensor.matmul(
                        pv[:],
                        xTc[k][mj][:, mo * 128 : (mo + 1) * 128],
                        wv_sb[:, k * NCH : (k + 1) * NCH],
                        start=(k == 0),
                        stop=(k == KT - 1),
                    )
                blk = v_t[m][:].rearrange("p (h c) -> p h c", c=65)
                nc.vector.tensor_copy(
                    blk[:, :, 0:64], pv[:].rearrange("p (h c) -> p h c", c=64)
                )
                nc.gpsimd.memset(blk[:, :, 64:65], 1.0)

            # The RoPE tail of each qk call (raw copy -> perm matmul -> rope
            # muls) is deferred into the NEXT qk call's emission slot: the
            # perm matmul would otherwise stall the in-order PE queue on the
            # Pool raw-copy; deferred, the copy overlaps the next call's
            # 8-matmul accumulation.
            qk_pending = []

            def qk_flush():
                while qk_pending:
                    qk_pending.pop(0)()

            def qk_proj(dst_t, w_sb, g, j):
                pp = ps.tile([128, 512], dt.float32, tag="ps", name="pp")
                for k in range(KT):
                    nc.tensor.matmul(
                        pp[:],
                        w_sb[:, k * NCH + g * 128 : k * NCH + (g + 1) * 128],
                        xTc[k][j][:],
                        start=(k == 0),
                        stop=(k == KT - 1),
                    )
                raw = wkp.tile([128, 512], dt.bfloat16, tag="raw")
                nc.scalar.copy(raw[:], pp[:])

                def rest():
                    # own PSUM slot (NOT pp): pp's slot is rewritten by the
                    # next call's accumulation, which sits earlier in the PE
                    # queue than this deferred matmul
                    pq = ps.tile([128, 512], dt.float32, tag="ps", name="pq")
                    nc.tensor.matmul(pq[:], perm_sb, raw[:], start=True, stop=True)
                    pqb = wkp.tile([128, 512], dt.bfloat16, tag="pqb")
                    nc.vector.tensor_copy(pqb[:], pq[:])
                    t1 = wkp.tile([128, 512], dt.bfloat16, tag="t1")
                    nc.vector.tensor_mul(
                        t1[:], raw[:], cos_sb[:, j * 512 : (j + 1) * 512]
                    )
                    t2 = wkp.tile([128, 512], dt.bfloat16, tag="t2")
                    nc.vector.tensor_mul(
                        t2[:], pqb[:], sin_sb[:, j * 512 : (j + 1) * 512]
                    )
                    nc.vector.tensor_add(dst_t[g][j][:], t1[:], t2[:])

                prev = qk_pending.pop(0) if qk_pending else None
                qk_pending.append(rest)
                if prev is not None:
                    prev()

            # ---- attention: sT = k_rot @ q_rot.T, exp, A@V with denominators ----
            # Head pairs (2g, 2g+1) interleaved block-by-block: their scores
            # matmuls are K=64 at base partitions 0/64, so the PE runs them
            # concurrently in disjoint row groups. Diagonal blocks narrow all
            # work to the causal column range [128r, 512).
            # Emitted as per-block closures so independent projection/wo
            # "filler" units can be zipped between blocks: engine queues run
            # in emission order, so the fillers soak up the PE bubbles left
            # by the ScalarE exp latency (~910ns/block vs PE ~740ns/block).
            def attention_closures(hp, j):
                g = hp
                nblk = 4 * j + 4
                pos = []
                pending = []

                def emit_av(e, lo, i):
                    for t in range(2):
                        h = 2 * hp + t
                        po = pos[t]
                        nc.tensor.matmul(
                            po[0:65, lo:512],
                            v_t[i][:, h * 65 : (h + 1) * 65],
                            e[:, t * 512 + lo : (t + 1) * 512],
                            start=(i == 0),
                            stop=(i == nblk - 1),
                        )

                def block(i):
                    if i == 0:
                        pos.extend(
                            po_p.tile([65, 512], dt.float32, tag="po", name=f"po{t}")
                            for t in range(2)
                        )
                    r = i - 4 * j
                    lo = 128 * r if r > 0 else 0
                    e = ep.tile([128, 1024], dt.bfloat16, tag="e")
                    # one exp per block (not per half): ScalarE cost is
                    # per-instruction overhead + width, and with the AV
                    # deferral below its latency is off the critical path
                    psw = scr_p.tile([128, 1024], dt.float32, tag="pscr", name="psw")
                    for t in range(2):
                        off = 64 * t
                        nc.tensor.matmul(
                            psw[:, t * 512 + lo : (t + 1) * 512],
                            k_t[g][i // 4][off : off + 64, (i % 4) * 128 : (i % 4 + 1) * 128],
                            q_t[g][j][off : off + 64, lo:512],
                            start=True,
                            stop=True,
                        )
                    if lo == 0:
                        nc.scalar.activation(e[:], psw[:], EXP, scale=0.125)
                    else:
                        src_ap = psw[:].rearrange("p (t c) -> p t c", t=2)[:, :, lo:512]
                        dst_ap = e[:].rearrange("p (t c) -> p t c", t=2)[:, :, lo:512]
                        nc.scalar.activation(dst_ap, src_ap, EXP, scale=0.125)
                    if r >= 0:
                        for t in range(2):
                            nc.gpsimd.tensor_mul(
                                e[:, t * 512 + lo : t * 512 + lo + 128],
                                e[:, t * 512 + lo : t * 512 + lo + 128],
                                tri_sb,
                            )
                    # defer this block's AV by one block: the PE queue is
                    # in-order, so emitting AV(i) right after scores(i)
                    # would stall the queue on exp(i); AV(i-1)'s exp is
                    # already done by now
                    prev = pending.pop() if pending else None
                    pending.append((e, lo, i))
                    if prev is not None:
                        emit_av(*prev)

                def flush():
                    emit_av(*pending.pop())

                # normalize: out_h = po[0:64] * broadcast(1 / po[64]). The
                # broadcast uses a single bf16 ones-matmul: attn_t is bf16
                # anyway, so an fp32-exact hi+lo broadcast buys nothing.
                # Split in two so the PE/DVE stage (norm_b) can be emitted a
                # few closures later, past the DVE reciprocal latency.
                rhis = []
                us = []

                def norm_a():
                    for t in range(2):
                        po = pos[t]
                        rec = sm.tile([1, 512], dt.float32, tag="rec")
                        nc.vector.reciprocal(rec[:], po[64:65, :])
                        rhi = sm.tile([1, 512], dt.bfloat16, tag="rhi")
                        nc.vector.tensor_copy(rhi[:], rec[:])
                        rhis.append(rhi)
                        # stage po in SBUF: walrus rejects TensorTensor with
                        # two PSUM operands
                        u_sb = sm.tile([64, 512], dt.float32, tag="u")
                        nc.vector.tensor_copy(u_sb[:], po[0:64, :])
                        us.append(u_sb)

                def norm_b():
                    for t in range(2):
                        off = 64 * t
                        pb = ps.tile([64, 512], dt.float32, tag="ps", name="pb")
                        nc.tensor.matmul(
                            pb[:], ones_sb[:], rhis[t][:], start=True, stop=True
                        )
                        nc.vector.tensor_mul(
                            attn_t[g][j][off : off + 64, :], us[t][:], pb[:]
                        )

                closures = [lambda i=i: block(i) for i in range(nblk)] + [flush, norm_a]
                return closures, norm_b

            def wo_proj(m):
                osb = ob.tile([128, 1024], dt.float32, tag="osb")
                for n in range(2):
                    pf = ps.tile([128, 512], dt.float32, tag="ps", name="pf")
                    for g in range(2):
                        nc.tensor.matmul(
                            pf[:],
                            attn_t[g][m // 4][:, (m % 4) * 128 : (m % 4 + 1) * 128],
                            wo_sb[:, g * D + n * 512 : g * D + (n + 1) * 512],
                            start=(g == 0),
                            stop=(g == 1),
                        )
                    nc.vector.tensor_copy(osb[:, n * 512 : (n + 1) * 512], pf[:])
                nc.sync.dma_start(out[m * 128 : (m + 1) * 128, :], osb[:])

            # ---- emission order sets scheduler priority ----
            # Prologue: chunk-0 projections (first data to land).
            for m in range(4):
                v_proj(m)
            for g in range(2):
                qk_proj(q_t, wq_sb, g, 0)
                qk_proj(k_t, wk_sb, g, 0)
            # Main: per chunk j, attention blocks zipped with filler units
            # (next chunk's v/qk projections, previous chunk's wo).
            SEC = [0, 1, 2, 3]
            normb_pending = []
            for si, j in enumerate(SEC):
                a0, nb0 = attention_closures(0, j)
                a1, nb1 = attention_closures(1, j)
                # nb0 goes right after a1's first block: past the reciprocal
                # latency, but before a1's first (deferred) AV write claims
                # a0's po PSUM slots
                blocks = (
                    [qk_flush] + normb_pending + a0 + [a1[0], nb0] + a1[1:]
                )
                normb_pending = [nb1]
                fillers = []
                if si + 1 < len(SEC):
                    jn = SEC[si + 1]
                    for m in range(4 * jn, 4 * jn + 4):
                        fillers.append(lambda m=m: v_proj(m))
                    for g in range(2):
                        fillers.append(lambda g=g: qk_proj(q_t, wq_sb, g, jn))
                        fillers.append(lambda g=g: qk_proj(k_t, wk_sb, g, jn))
                if si > 0:
                    jp = SEC[si - 1]
                    for m in range(4 * jp, 4 * jp + 4):
                        fillers.append(lambda m=m: wo_proj(m))
                stride = max(1, (len(blocks) + len(fillers)) // max(1, len(fillers)))
                fi = 0
                for bi, b in enumerate(blocks):
                    b()
                    if (bi + 1) % stride == 0 and fi < len(fillers):
                        fillers[fi]()
                        fi += 1
                while fi < len(fillers):
                    fillers[fi]()
                    fi += 1
            for nb in normb_pending:
                nb()
            for m in range(4 * SEC[-1], 4 * SEC[-1] + 4):
                wo_proj(m)

    _split_multi_waits(nc)
    return nc


def _sbuf_layout(a128xN):
    # (T*128, N) -> (128, T*N) with tile t at columns [t*N, (t+1)*N)
    t = a128xN.shape[0] // 128
    n = a128xN.shape[1]
    return np.ascontiguousarray(
        a128xN.reshape(t, 128, n).transpose(1, 0, 2).reshape(128, t * n)
    )


def _host_prep(x, wq, wk, wv, wo, token_positions):
    x = np.asarray(x, dtype=np.float32)
    wq = np.asarray(wq, dtype=np.float32)
    wk = np.asarray(wk, dtype=np.float32)
    wv = np.asarray(wv, dtype=np.float32)
    wo = np.asarray(wo, dtype=np.float32)
    pos = np.asarray(token_positions).astype(np.float32)

    # deinterleave channel order within each head for q/k: [evens, odds]
    de = np.concatenate([np.arange(0, HD, 2), np.arange(1, HD, 2)])

    # RoPE tables, extended to the 128-partition tile layout
    inv_freq = (1.0 / (THETA ** (np.arange(0, HD, 2, dtype=np.float32) / HD))).astype(
        np.float32
    )
    freqs = pos[:, None] * inv_freq[None, :]  # (S, 32)
    cosT = np.cos(freqs).astype(np.float32).T  # (32, S)
    sinT = np.sin(freqs).astype(np.float32).T
    cos_l = np.ascontiguousarray(np.tile(cosT, (4, 1)))  # (128, S)
    sin_l = np.ascontiguousarray(
        np.concatenate([-sinT, sinT, -sinT, sinT], axis=0)
    )

    # 128x128 half-swap permutation (block diag of two 64-blocks)
    p64 = np.zeros((64, 64), np.float32)
    for i in range(64):
        p64[i, (i + 32) % 64] = 1.0
    perm_l = np.zeros((128, 128), np.float32)
    perm_l[:64, :64] = p64
    perm_l[64:, 64:] = p64

    tri_l = (np.arange(128)[None, :] >= np.arange(128)[:, None]).astype(np.float32)

    in_maps = []
    for c in range(8):
        b, hg = divmod(c, 4)
        rows = hg * NCH + np.arange(NCH)
        # per-head deinterleave for q/k channel rows
        rows_de = (rows.reshape(HPC, HD)[:, de]).reshape(-1)

        xT = np.ascontiguousarray(x[b].T)  # (D, S)
        wq_t = np.ascontiguousarray(wq[rows_de, :].T)  # (D, 256)
        wk_t = np.ascontiguousarray(wk[rows_de, :].T)
        wv_t = np.ascontiguousarray(wv[rows, :].T)
        wo_t = np.ascontiguousarray(wo[:, rows].T)  # (256, D)

        wpk = np.concatenate(
            [
                _sbuf_layout(wv_t),
                _sbuf_layout(wq_t),
                _sbuf_layout(wk_t),
                perm_l,
                tri_l,
                _sbuf_layout(wo_t),
            ],
            axis=1,
        ).astype(BF16)
        fpk = np.concatenate(
            [cos_l, sin_l, np.zeros((128, KVER), np.float32)], axis=1
        ).astype(BF16)
        in_maps.append({"xT": xT.astype(BF16), "wpack": wpk, "fpack": fpk})
    return in_maps


def _get_nc():
    if "nc" not in _CACHE:
        _CACHE["nc"] = _build_nc()
    return _CACHE["nc"]


def kernel(x, wq, wk, wv, wo, token_positions, _trace=False, _tmpdir=None):
    nc = _get_nc()
    in_maps = _host_prep(x, wq, wk, wv, wo, token_positions)
    res = run_bass_kernel_spmd(
        nc, in_maps, core_ids=list(range(8)), trace=_trace, tmpdir=_tmpdir
    )
    out = np.zeros((B, S, D), np.float32)
    for c in range(8):
        b = c // 4
        out[b] += res.results[c]["o"]
    if _trace:
        kernel._last_result = res
    return out
